# revision 18
# baseline (speedup 1.0000x reference)
"""Trainium2 Bass kernel for Exercise-KC GraphConvolution (concat=True branch).

Computes: elu((adj @ (kc_h @ W1)) * (ex_h @ W1 @ W2))   -> [50000, 512]

Strategy (8 NeuronCores):
  - Shard exercise rows across cores: pad 50000 -> 50176 = 8 * 49 * 128.
  - Batch-independent weight folding on host: kcWh = kc_h @ W1 ([2048, 512])
    and W12 = W1 @ W2 ([512, 512]) are precomputed in fp32 and shipped to the
    device, removing all setup matmuls from the kernel.
  - Everything streams in fp16 (measured end-to-end rel err ~5e-4 vs the
    2e-2 gate): half the HBM traffic of fp32/fp32r at the same PE rate
    (1 col/cycle), and FWL (fast weight load) fully hides LDWEIGHTS.
  - DMA issue discipline: every dma_start costs ~0.6-0.7us on the issuing
    engine and round-robins over 8 completion-semaphore lanes (an issue
    waits for the 8-back DMA).  So the startup working set moves in exactly
    8 batched DMAs (ex-group + w12 + kcWh halves + first 4 adj tiles), and
    steady-state ex chunks are batched 4 tiles per DMA.  PSUM-freeing exb
    copies run on the vector engine so the scalar engine's DMA issues never
    delay them.
  - The exercise branches of the first 4 tiles run during the startup window
    (they need only W12 + one small ex-group DMA), parking ex_h @ W12 in
    SBUF as fp16; the spmm phase starts as kcWh lands.
  - Per 128-row output tile: 4 accumulating matmuls for the exercise branch
    (K=512) then 16 for the spmm (K=2048), all N=512, then
    elu(x) = max(x, exp(min(x,0)) - 1) on vector+scalar engines.
  - Last tile is split into two N=256 halves so the final elementwise chain
    and store overlap the last spmm matmuls (shorter kernel tail).
"""

import numpy as np

import concourse.bass as bass
import concourse.mybir as mybir
import concourse.tile as tile
from concourse import bacc
from concourse.bass_utils import run_bass_kernel_spmd

N_EX = 50000
IN_F = 512
OUT_F = 512
N_KC = 2048
N_CORES = 8

P = 128                       # partitions
T = 49                        # row-tiles per core
E_PER_CORE = T * P            # 6272
E_PAD = N_CORES * E_PER_CORE  # 50176
KHI_ADJ = N_KC // P           # 16
KHI_IN = IN_F // P            # 4
FB = OUT_F                    # 512 (psum free dim)

EXG = 4                       # ex-chunk tiles batched per DMA
N_EXG = (T + EXG - 1) // EXG  # 13 groups (last one padded)
ADJ_PRE = 4                   # adjacency tiles DMA'd at startup

F32 = mybir.dt.float32
F16 = mybir.dt.float16


def build_nc(n_tiles: int = T):
    """Build + compile the per-core Bass program (same program on all cores)."""
    nc = bacc.Bacc(
        "TRN2",
        target_bir_lowering=False,
        debug=False,
        enable_asserts=False,
        num_devices=N_CORES,
    )
    AF = mybir.ActivationFunctionType
    OP = mybir.AluOpType

    adjs = nc.dram_tensor("adjs", [n_tiles, P, KHI_ADJ, P], F16,
                          kind="ExternalInput")
    # ex chunks grouped EXG tiles per DMA: [group, k_lo, tile-in-group, kj, m]
    exsg = nc.dram_tensor("exsg", [N_EXG, P, EXG, KHI_IN, P], F16,
                          kind="ExternalInput")
    # kcWh = kc_h @ W1 (host-folded), partition-major [k_lo][ki][n]
    kcw = nc.dram_tensor("kcw", [P, KHI_ADJ, FB], F16, kind="ExternalInput")
    # W12 = W1 @ W2 (host-folded), k-major [k_lo][kj][n]
    w12 = nc.dram_tensor("w12", [P, KHI_IN, FB], F16, kind="ExternalInput")
    outp = nc.dram_tensor("outp", [n_tiles, P, FB], F16, kind="ExternalOutput")

    def ring(i):
        return nc.sync if i % 2 == 0 else nc.scalar

    with tile.TileContext(nc) as tc:
        with (
            tc.tile_pool(name="const", bufs=1) as constp,
            tc.tile_pool(name="adj", bufs=6) as adjp,
            tc.tile_pool(name="exg", bufs=4) as exgp,
            tc.tile_pool(name="exb", bufs=2 * EXG + 4) as exbp,
            tc.tile_pool(name="outb", bufs=3) as outbp,
            tc.tile_pool(name="tmp", bufs=3) as tmpp,
            tc.tile_pool(name="ps", bufs=2, space=bass.MemorySpace.PSUM) as psp,
            tc.tile_pool(name="psq", bufs=2, space=bass.MemorySpace.PSUM) as psq,
        ):
            # PE warm-up: the HAM clock gate needs ~3.4us of activity to lift
            # the 1.2GHz cold throttle; burn it on a zero tile while the
            # startup DMAs are in flight.
            warm_sb = constp.tile([P, P], F16)
            nc.vector.memset(warm_sb[:], 0.0)
            for _ in range(44):
                pwu = psq.tile([P, P], F32, tag="pw")
                nc.tensor.matmul(pwu[:], warm_sb[:], warm_sb[:],
                                 start=True, stop=True)

            # --- startup: 9 DMA issues, arrival-ordered so the PE's
            # dependency chain is fed just in time: ex groups 0/1 + w12
            # first (8 tiles of ex-branch work), kcWh halves next, the
            # first adjacency tiles after.
            ex_gs = {}
            eg0 = exgp.tile([P, EXG, KHI_IN, P], F16, tag="exg")
            ex_gs[0] = eg0
            nc.sync.dma_start(eg0[:], exsg[0])
            w12_sb = constp.tile([P, KHI_IN, FB], F16)
            nc.scalar.dma_start(w12_sb[:], w12[:])
            eg1 = exgp.tile([P, EXG, KHI_IN, P], F16, tag="exg")
            ex_gs[1] = eg1
            nc.scalar.dma_start(eg1[:], exsg[1])
            kcw_sb = constp.tile([P, KHI_ADJ, FB], F16)
            HK = KHI_ADJ // 2
            nc.sync.dma_start(kcw_sb[:, :HK, :], kcw[:, :HK, :])
            nc.scalar.dma_start(kcw_sb[:, HK:, :], kcw[:, HK:, :])
            adj_sbs = {}
            for t in range(ADJ_PRE):
                ag = adjp.tile([P, KHI_ADJ, P], F16, tag="adj")
                adj_sbs[t] = ag
            nc.sync.dma_start(adj_sbs[0][:], adjs[0])
            nc.scalar.dma_start(adj_sbs[1][:], adjs[1])
            nc.sync.dma_start(adj_sbs[2][:], adjs[2])
            nc.scalar.dma_start(adj_sbs[3][:], adjs[3])

            exb_sbs = {}

            def ex_branch(t):
                """ex_h @ W12 for tile t -> fp16 SBUF tile (via PSUM copy)."""
                eg = ex_gs[t // EXG]
                ps_e = psp.tile([P, FB], F32, tag="pse")
                for kj in range(KHI_IN):
                    nc.tensor.matmul(
                        ps_e[:],
                        eg[:, t % EXG, kj, :],
                        w12_sb[:, kj, :],
                        start=(kj == 0),
                        stop=(kj == KHI_IN - 1),
                    )
                # copy on the vector engine: the scalar engine's DMA issues
                # must never delay freeing the ex-branch PSUM buffer
                exb = exbp.tile([P, FB], F16)
                nc.vector.tensor_copy(exb[:], ps_e[:])
                exb_sbs[t] = exb

            def elementwise(t, ps_s, exb_ap, o_sb, n):
                # elu(prod) = max(prod, exp(min(prod,0)) - 1)
                prod = tmpp.tile([P, n], F32, tag=f"prod{n}")
                nc.vector.tensor_tensor(prod[:], ps_s[:], exb_ap, OP.mult)
                nmin = tmpp.tile([P, n], F32, tag=f"nmin{n}")
                nc.vector.tensor_scalar(nmin[:], prod[:], 0.0, None, OP.min)
                expv = tmpp.tile([P, n], F32, tag=f"expv{n}")
                nc.scalar.activation(expv[:], nmin[:], AF.Exp)
                nc.vector.scalar_tensor_tensor(
                    o_sb, expv[:], -1.0, prod[:], OP.add, OP.max)

            # ex branches of the first two groups run while kcWh is in flight
            for t in range(2 * EXG):
                ex_branch(t)

            # ---- main loop over row-tiles ----
            for t in range(n_tiles):
                if t >= 2 * EXG:
                    ex_branch(t)

                a_sb = adj_sbs.pop(t)
                exb = exb_sbs.pop(t)
                if t < n_tiles - 1:
                    ps_s = psp.tile([P, FB], F32, tag="pss")  # spmm branch
                    for ki in range(KHI_ADJ):
                        nc.tensor.matmul(
                            ps_s[:],
                            a_sb[:, ki, :],
                            kcw_sb[:, ki, :],
                            start=(ki == 0),
                            stop=(ki == KHI_ADJ - 1),
                        )
                    o_sb = outbp.tile([P, FB], F16)
                    elementwise(t, ps_s, exb[:], o_sb[:], FB)
                    # lookahead DMA issues BEFORE the store: their buffers
                    # are released tiles ago (pool depth > lookahead), so
                    # they issue immediately instead of queuing behind the
                    # store's wait on the elementwise chain
                    ta = t + ADJ_PRE
                    if ta < n_tiles:
                        ag = adjp.tile([P, KHI_ADJ, P], F16, tag="adj")
                        ring(ta).dma_start(ag[:], adjs[ta])
                        adj_sbs[ta] = ag
                    if t % EXG == 1:
                        g = t // EXG + 2
                        if g < (n_tiles + EXG - 1) // EXG:
                            eg = exgp.tile([P, EXG, KHI_IN, P], F16)
                            ring(g).dma_start(eg[:], exsg[g])
                            ex_gs[g] = eg
                    ring(t).dma_start(outp[t], o_sb[:])
                else:
                    # last tile: split N in half so the elementwise chain and
                    # store of half 0 overlap the spmm matmuls of half 1
                    HF = FB // 2
                    for h in range(2):
                        sl = slice(h * HF, (h + 1) * HF)
                        ps_h = psp.tile([P, HF], F32, tag="pslast")
                        for ki in range(KHI_ADJ):
                            nc.tensor.matmul(
                                ps_h[:],
                                a_sb[:, ki, :],
                                kcw_sb[:, ki, sl],
                                start=(ki == 0),
                                stop=(ki == KHI_ADJ - 1),
                            )
                        o_sb = outbp.tile([P, HF], F16, tag="olast")
                        elementwise(t, ps_h, exb[:, sl], o_sb[:], HF)
                        ring(h).dma_start(outp[t, :, sl], o_sb[:])

    nc.compile()
    return nc


def prep_inputs(exercise_h, kc_h, adj_exercise_kc, W1, W2,
                n_tiles: int = T):
    """Host-side shard + layout prep. Returns in_maps (one dict per core)."""
    ex = np.asarray(exercise_h, dtype=np.float32)
    kc = np.asarray(kc_h, dtype=np.float32)
    adj = np.asarray(adj_exercise_kc, dtype=np.float32)
    w1 = np.asarray(W1, dtype=np.float32)
    w2 = np.asarray(W2, dtype=np.float32)

    # batch-independent weight folding (exact fp32, then one fp16 rounding)
    kcwh = (kc @ w1).astype(np.float16)                    # [2048, 512]
    w12 = (w1 @ w2).astype(np.float16)                     # [512, 512]

    e_pad = N_CORES * n_tiles * P
    n_rows = min(N_EX, e_pad)

    adj_p = np.zeros((e_pad, N_KC), np.float16)
    adj_p[:n_rows] = adj[:n_rows]
    n_exg = (n_tiles + EXG - 1) // EXG
    ex_p = np.zeros((N_CORES * n_exg * EXG * P, IN_F), np.float16)
    # per-core row blocks padded to the group grid
    exf = np.asarray(ex[:n_rows], np.float16)
    for c in range(N_CORES):
        lo, hi = c * E_PER_CORE, min((c + 1) * E_PER_CORE, n_rows)
        ex_p[c * n_exg * EXG * P: c * n_exg * EXG * P + (hi - lo)] = \
            exf[lo:hi]
    # [core, t, k_lo, k_hi, m] (k-major chunks, stationary-operand layout)
    adjs = np.ascontiguousarray(
        adj_p.reshape(N_CORES, n_tiles, P, KHI_ADJ, P).transpose(0, 1, 4, 3, 2))
    # [core, group, k_lo, tile-in-group, kj, m]
    exsg = np.ascontiguousarray(
        ex_p.reshape(N_CORES, n_exg, EXG, P, KHI_IN, P)
        .transpose(0, 1, 5, 2, 4, 3))

    kcw = np.ascontiguousarray(
        kcwh.reshape(KHI_ADJ, P, FB).transpose(1, 0, 2))
    w12r = np.ascontiguousarray(w12.reshape(KHI_IN, P, FB).transpose(1, 0, 2))

    return [
        {"adjs": adjs[c], "exsg": exsg[c], "kcw": kcw, "w12": w12r}
        for c in range(N_CORES)
    ]


def unpack_output(results, n_tiles: int = T) -> np.ndarray:
    """results: list per core of {"outp": [n_tiles, P, FB]} -> [N_EX, FB]."""
    per_core = [
        np.asarray(r["outp"]).reshape(n_tiles * P, FB)
        for r in results
    ]
    return np.concatenate(per_core, axis=0)[:N_EX].astype(np.float32)


_NC_CACHE: dict = {}


def _get_nc():
    if T not in _NC_CACHE:
        _NC_CACHE[T] = build_nc()
    return _NC_CACHE[T]


def kernel(exercise_h, kc_h, adj_exercise_kc, W1, W2):
    nc = _get_nc()
    in_maps = prep_inputs(exercise_h, kc_h, adj_exercise_kc, W1, W2)
    res = run_bass_kernel_spmd(nc, in_maps, core_ids=list(range(N_CORES)))
    return np.ascontiguousarray(unpack_output(res.results))


# revision 19
# speedup vs baseline: 1.0075x; 1.0075x over previous
"""Trainium2 Bass kernel for Exercise-KC GraphConvolution (concat=True branch).

Computes: elu((adj @ (kc_h @ W1)) * (ex_h @ W1 @ W2))   -> [50000, 512]

Strategy (8 NeuronCores):
  - Shard exercise rows across cores: pad 50000 -> 50176 = 8 * 49 * 128.
  - Batch-independent weight folding on host: kcWh = kc_h @ W1 ([2048, 512])
    and W12 = W1 @ W2 ([512, 512]) are precomputed in fp32 and shipped to the
    device, removing all setup matmuls from the kernel.
  - Everything streams in fp16 (measured end-to-end rel err ~5e-4 vs the
    2e-2 gate): half the HBM traffic of fp32/fp32r at the same PE rate
    (1 col/cycle), and FWL (fast weight load) fully hides LDWEIGHTS.
  - DMA issue discipline: every dma_start costs ~0.6-0.7us on the issuing
    engine and round-robins over 8 completion-semaphore lanes (an issue
    waits for the 8-back DMA).  So the startup working set moves in exactly
    8 batched DMAs (ex-group + w12 + kcWh halves + first 4 adj tiles), and
    steady-state ex chunks are batched 4 tiles per DMA.  PSUM-freeing exb
    copies run on the vector engine so the scalar engine's DMA issues never
    delay them.
  - The exercise branches of the first 4 tiles run during the startup window
    (they need only W12 + one small ex-group DMA), parking ex_h @ W12 in
    SBUF as fp16; the spmm phase starts as kcWh lands.
  - Per 128-row output tile: 4 accumulating matmuls for the exercise branch
    (K=512) then 16 for the spmm (K=2048), all N=512, then
    elu(x) = max(x, exp(min(x,0)) - 1) on vector+scalar engines.
  - Last tile is split into two N=256 halves so the final elementwise chain
    and store overlap the last spmm matmuls (shorter kernel tail).
"""

import numpy as np

import concourse.bass as bass
import concourse.mybir as mybir
import concourse.tile as tile
from concourse import bacc
from concourse.bass_utils import run_bass_kernel_spmd

N_EX = 50000
IN_F = 512
OUT_F = 512
N_KC = 2048
N_CORES = 8

P = 128                       # partitions
T = 49                        # row-tiles per core
E_PER_CORE = T * P            # 6272
E_PAD = N_CORES * E_PER_CORE  # 50176
KHI_ADJ = N_KC // P           # 16
KHI_IN = IN_F // P            # 4
FB = OUT_F                    # 512 (psum free dim)

EXG = 4                       # ex-chunk tiles batched per DMA
N_EXG = (T + EXG - 1) // EXG  # 13 groups (last one padded)
ADJ_PRE = 4                   # adjacency tiles DMA'd at startup

F32 = mybir.dt.float32
F16 = mybir.dt.float16


def build_nc(n_tiles: int = T):
    """Build + compile the per-core Bass program (same program on all cores)."""
    nc = bacc.Bacc(
        "TRN2",
        target_bir_lowering=False,
        debug=False,
        enable_asserts=False,
        num_devices=N_CORES,
    )
    AF = mybir.ActivationFunctionType
    OP = mybir.AluOpType

    adjs = nc.dram_tensor("adjs", [n_tiles, P, KHI_ADJ, P], F16,
                          kind="ExternalInput")
    # ex chunks grouped EXG tiles per DMA: [group, k_lo, tile-in-group, kj, m]
    exsg = nc.dram_tensor("exsg", [N_EXG, P, EXG, KHI_IN, P], F16,
                          kind="ExternalInput")
    # kcWh = kc_h @ W1 (host-folded), partition-major [k_lo][ki][n]
    kcw = nc.dram_tensor("kcw", [P, KHI_ADJ, FB], F16, kind="ExternalInput")
    # W12 = W1 @ W2 (host-folded), k-major [k_lo][kj][n]
    w12 = nc.dram_tensor("w12", [P, KHI_IN, FB], F16, kind="ExternalInput")
    outp = nc.dram_tensor("outp", [n_tiles, P, FB], F16, kind="ExternalOutput")

    def ring(i):
        return nc.sync if i % 2 == 0 else nc.scalar

    with tile.TileContext(nc) as tc:
        with (
            tc.tile_pool(name="const", bufs=1) as constp,
            tc.tile_pool(name="adj", bufs=6) as adjp,
            tc.tile_pool(name="exg", bufs=4) as exgp,
            tc.tile_pool(name="exb", bufs=2 * EXG + 4) as exbp,
            tc.tile_pool(name="outb", bufs=3) as outbp,
            tc.tile_pool(name="tmp", bufs=3) as tmpp,
            tc.tile_pool(name="ps", bufs=2, space=bass.MemorySpace.PSUM) as psp,
            tc.tile_pool(name="psq", bufs=2, space=bass.MemorySpace.PSUM) as psq,
        ):
            # PE warm-up: the HAM clock gate needs ~3.4us of activity to lift
            # the 1.2GHz cold throttle; burn it on a zero tile while the
            # startup DMAs are in flight.
            # single accumulation group: no inter-MM completion semaphores,
            # so warmup matmuls run back-to-back at the issue rate
            warm_sb = constp.tile([P, P], F16)
            nc.vector.memset(warm_sb[:], 0.0)
            N_WARM = 44
            pwu = psq.tile([P, P], F32, tag="pw")
            for i in range(N_WARM):
                nc.tensor.matmul(pwu[:], warm_sb[:], warm_sb[:],
                                 start=(i == 0), stop=(i == N_WARM - 1))

            # --- startup DMAs in PE-consumption order, kcWh in quarters,
            # so every transfer lands just before its first consumer:
            # [w12, exg0] -> ex0-3, [exg1, adj0] -> ex4-7/spmm0,
            # [kcwQ1..Q4] -> spmm0 k-chunks, [adj1..adj3] -> spmm1-3.
            ex_gs = {}
            eg0 = exgp.tile([P, EXG, KHI_IN, P], F16, tag="exg")
            ex_gs[0] = eg0
            nc.sync.dma_start(eg0[:], exsg[0])
            w12_sb = constp.tile([P, KHI_IN, FB], F16)
            nc.scalar.dma_start(w12_sb[:], w12[:])
            eg1 = exgp.tile([P, EXG, KHI_IN, P], F16, tag="exg")
            ex_gs[1] = eg1
            nc.sync.dma_start(eg1[:], exsg[1])
            adj_sbs = {}
            for t in range(ADJ_PRE):
                ag = adjp.tile([P, KHI_ADJ, P], F16, tag="adj")
                adj_sbs[t] = ag
            nc.scalar.dma_start(adj_sbs[0][:], adjs[0])
            kcw_sb = constp.tile([P, KHI_ADJ, FB], F16)
            QK = KHI_ADJ // 4
            for q in range(4):
                sl = slice(q * QK, (q + 1) * QK)
                ring(q).dma_start(kcw_sb[:, sl, :], kcw[:, sl, :])
            nc.sync.dma_start(adj_sbs[1][:], adjs[1])
            nc.scalar.dma_start(adj_sbs[2][:], adjs[2])
            nc.sync.dma_start(adj_sbs[3][:], adjs[3])

            exb_sbs = {}

            def ex_branch(t):
                """ex_h @ W12 for tile t -> fp16 SBUF tile (via PSUM copy)."""
                eg = ex_gs[t // EXG]
                ps_e = psp.tile([P, FB], F32, tag="pse")
                for kj in range(KHI_IN):
                    nc.tensor.matmul(
                        ps_e[:],
                        eg[:, t % EXG, kj, :],
                        w12_sb[:, kj, :],
                        start=(kj == 0),
                        stop=(kj == KHI_IN - 1),
                    )
                # copy on the vector engine: the scalar engine's DMA issues
                # must never delay freeing the ex-branch PSUM buffer
                exb = exbp.tile([P, FB], F16)
                nc.vector.tensor_copy(exb[:], ps_e[:])
                exb_sbs[t] = exb

            def elementwise(t, ps_s, exb_ap, o_sb, n):
                # elu(prod) = max(prod, exp(min(prod,0)) - 1)
                prod = tmpp.tile([P, n], F32, tag=f"prod{n}")
                nc.vector.tensor_tensor(prod[:], ps_s[:], exb_ap, OP.mult)
                nmin = tmpp.tile([P, n], F32, tag=f"nmin{n}")
                nc.vector.tensor_scalar(nmin[:], prod[:], 0.0, None, OP.min)
                expv = tmpp.tile([P, n], F32, tag=f"expv{n}")
                nc.scalar.activation(expv[:], nmin[:], AF.Exp)
                nc.vector.scalar_tensor_tensor(
                    o_sb, expv[:], -1.0, prod[:], OP.add, OP.max)

            # ex branches of the first two groups run while kcWh is in flight
            for t in range(2 * EXG):
                ex_branch(t)

            # ---- main loop over row-tiles ----
            for t in range(n_tiles):
                if t >= 2 * EXG:
                    ex_branch(t)

                a_sb = adj_sbs.pop(t)
                exb = exb_sbs.pop(t)
                if t < n_tiles - 1:
                    ps_s = psp.tile([P, FB], F32, tag="pss")  # spmm branch
                    for ki in range(KHI_ADJ):
                        nc.tensor.matmul(
                            ps_s[:],
                            a_sb[:, ki, :],
                            kcw_sb[:, ki, :],
                            start=(ki == 0),
                            stop=(ki == KHI_ADJ - 1),
                        )
                    o_sb = outbp.tile([P, FB], F16)
                    elementwise(t, ps_s, exb[:], o_sb[:], FB)
                    # lookahead DMA issues BEFORE the store: their buffers
                    # are released tiles ago (pool depth > lookahead), so
                    # they issue immediately instead of queuing behind the
                    # store's wait on the elementwise chain
                    ta = t + ADJ_PRE
                    if ta < n_tiles:
                        ag = adjp.tile([P, KHI_ADJ, P], F16, tag="adj")
                        ring(ta).dma_start(ag[:], adjs[ta])
                        adj_sbs[ta] = ag
                    if t % EXG == 1:
                        g = t // EXG + 2
                        if g < (n_tiles + EXG - 1) // EXG:
                            eg = exgp.tile([P, EXG, KHI_IN, P], F16)
                            ring(g).dma_start(eg[:], exsg[g])
                            ex_gs[g] = eg
                    ring(t).dma_start(outp[t], o_sb[:])
                else:
                    # last tile: split N in half so the elementwise chain and
                    # store of half 0 overlap the spmm matmuls of half 1
                    HF = FB // 2
                    for h in range(2):
                        sl = slice(h * HF, (h + 1) * HF)
                        ps_h = psp.tile([P, HF], F32, tag="pslast")
                        for ki in range(KHI_ADJ):
                            nc.tensor.matmul(
                                ps_h[:],
                                a_sb[:, ki, :],
                                kcw_sb[:, ki, sl],
                                start=(ki == 0),
                                stop=(ki == KHI_ADJ - 1),
                            )
                        o_sb = outbp.tile([P, HF], F16, tag="olast")
                        elementwise(t, ps_h, exb[:, sl], o_sb[:], HF)
                        ring(h).dma_start(outp[t, :, sl], o_sb[:])

    nc.compile()
    return nc


def prep_inputs(exercise_h, kc_h, adj_exercise_kc, W1, W2,
                n_tiles: int = T):
    """Host-side shard + layout prep. Returns in_maps (one dict per core)."""
    ex = np.asarray(exercise_h, dtype=np.float32)
    kc = np.asarray(kc_h, dtype=np.float32)
    adj = np.asarray(adj_exercise_kc, dtype=np.float32)
    w1 = np.asarray(W1, dtype=np.float32)
    w2 = np.asarray(W2, dtype=np.float32)

    # batch-independent weight folding (exact fp32, then one fp16 rounding)
    kcwh = (kc @ w1).astype(np.float16)                    # [2048, 512]
    w12 = (w1 @ w2).astype(np.float16)                     # [512, 512]

    e_pad = N_CORES * n_tiles * P
    n_rows = min(N_EX, e_pad)

    adj_p = np.zeros((e_pad, N_KC), np.float16)
    adj_p[:n_rows] = adj[:n_rows]
    n_exg = (n_tiles + EXG - 1) // EXG
    ex_p = np.zeros((N_CORES * n_exg * EXG * P, IN_F), np.float16)
    # per-core row blocks padded to the group grid
    exf = np.asarray(ex[:n_rows], np.float16)
    for c in range(N_CORES):
        lo, hi = c * E_PER_CORE, min((c + 1) * E_PER_CORE, n_rows)
        ex_p[c * n_exg * EXG * P: c * n_exg * EXG * P + (hi - lo)] = \
            exf[lo:hi]
    # [core, t, k_lo, k_hi, m] (k-major chunks, stationary-operand layout)
    adjs = np.ascontiguousarray(
        adj_p.reshape(N_CORES, n_tiles, P, KHI_ADJ, P).transpose(0, 1, 4, 3, 2))
    # [core, group, k_lo, tile-in-group, kj, m]
    exsg = np.ascontiguousarray(
        ex_p.reshape(N_CORES, n_exg, EXG, P, KHI_IN, P)
        .transpose(0, 1, 5, 2, 4, 3))

    kcw = np.ascontiguousarray(
        kcwh.reshape(KHI_ADJ, P, FB).transpose(1, 0, 2))
    w12r = np.ascontiguousarray(w12.reshape(KHI_IN, P, FB).transpose(1, 0, 2))

    return [
        {"adjs": adjs[c], "exsg": exsg[c], "kcw": kcw, "w12": w12r}
        for c in range(N_CORES)
    ]


def unpack_output(results, n_tiles: int = T) -> np.ndarray:
    """results: list per core of {"outp": [n_tiles, P, FB]} -> [N_EX, FB]."""
    per_core = [
        np.asarray(r["outp"]).reshape(n_tiles * P, FB)
        for r in results
    ]
    return np.concatenate(per_core, axis=0)[:N_EX].astype(np.float32)


_NC_CACHE: dict = {}


def _get_nc():
    if T not in _NC_CACHE:
        _NC_CACHE[T] = build_nc()
    return _NC_CACHE[T]


def kernel(exercise_h, kc_h, adj_exercise_kc, W1, W2):
    nc = _get_nc()
    in_maps = prep_inputs(exercise_h, kc_h, adj_exercise_kc, W1, W2)
    res = run_bass_kernel_spmd(nc, in_maps, core_ids=list(range(N_CORES)))
    return np.ascontiguousarray(unpack_output(res.results))


# revision 20
# speedup vs baseline: 1.0184x; 1.0108x over previous
"""Trainium2 Bass kernel for Exercise-KC GraphConvolution (concat=True branch).

Computes: elu((adj @ (kc_h @ W1)) * (ex_h @ W1 @ W2))   -> [50000, 512]

Strategy (8 NeuronCores):
  - Shard exercise rows across cores: pad 50000 -> 50176 = 8 * 49 * 128.
  - Batch-independent weight folding on host: kcWh = kc_h @ W1 ([2048, 512])
    and W12 = W1 @ W2 ([512, 512]) are precomputed in fp32 and shipped to the
    device, removing all setup matmuls from the kernel.
  - Everything streams in fp16 (measured end-to-end rel err ~5e-4 vs the
    2e-2 gate): half the HBM traffic of fp32/fp32r at the same PE rate
    (1 col/cycle), and FWL (fast weight load) fully hides LDWEIGHTS.
  - DMA issue discipline: every dma_start costs ~0.6-0.7us on the issuing
    engine and round-robins over 8 completion-semaphore lanes (an issue
    waits for the 8-back DMA).  So the startup working set moves in exactly
    8 batched DMAs (ex-group + w12 + kcWh halves + first 4 adj tiles), and
    steady-state ex chunks are batched 4 tiles per DMA.  PSUM-freeing exb
    copies run on the vector engine so the scalar engine's DMA issues never
    delay them.
  - The exercise branches of the first 4 tiles run during the startup window
    (they need only W12 + one small ex-group DMA), parking ex_h @ W12 in
    SBUF as fp16; the spmm phase starts as kcWh lands.
  - Per 128-row output tile: 4 accumulating matmuls for the exercise branch
    (K=512) then 16 for the spmm (K=2048), all N=512, then
    elu(x) = max(x, exp(min(x,0)) - 1) on vector+scalar engines.
  - Last tile is split into two N=256 halves so the final elementwise chain
    and store overlap the last spmm matmuls (shorter kernel tail).
"""

import numpy as np

import concourse.bass as bass
import concourse.mybir as mybir
import concourse.tile as tile
from concourse import bacc
from concourse.bass_utils import run_bass_kernel_spmd

N_EX = 50000
IN_F = 512
OUT_F = 512
N_KC = 2048
N_CORES = 8

P = 128                       # partitions
T = 49                        # row-tiles per core
E_PER_CORE = T * P            # 6272
E_PAD = N_CORES * E_PER_CORE  # 50176
KHI_ADJ = N_KC // P           # 16
KHI_IN = IN_F // P            # 4
FB = OUT_F                    # 512 (psum free dim)

EXG = 4                       # ex-chunk tiles batched per DMA
N_EXG = (T + EXG - 1) // EXG  # 13 groups (last one padded)
ADJ_PRE = 4                   # adjacency tiles DMA'd at startup

F32 = mybir.dt.float32
F16 = mybir.dt.float16


def build_nc(n_tiles: int = T):
    """Build + compile the per-core Bass program (same program on all cores)."""
    nc = bacc.Bacc(
        "TRN2",
        target_bir_lowering=False,
        debug=False,
        enable_asserts=False,
        num_devices=N_CORES,
    )
    AF = mybir.ActivationFunctionType
    OP = mybir.AluOpType

    adjs = nc.dram_tensor("adjs", [n_tiles, P, KHI_ADJ, P], F16,
                          kind="ExternalInput")
    # ex chunks grouped EXG tiles per DMA: [group, k_lo, tile-in-group, kj, m]
    exsg = nc.dram_tensor("exsg", [N_EXG, P, EXG, KHI_IN, P], F16,
                          kind="ExternalInput")
    # kcWh = kc_h @ W1 (host-folded), partition-major [k_lo][ki][n]
    kcw = nc.dram_tensor("kcw", [P, KHI_ADJ, FB], F16, kind="ExternalInput")
    # W12 = W1 @ W2 (host-folded), k-major [k_lo][kj][n]
    w12 = nc.dram_tensor("w12", [P, KHI_IN, FB], F16, kind="ExternalInput")
    outp = nc.dram_tensor("outp", [n_tiles, P, FB], F16, kind="ExternalOutput")

    def ring(i):
        return nc.sync if i % 2 == 0 else nc.scalar

    with tile.TileContext(nc) as tc:
        with (
            tc.tile_pool(name="const", bufs=1) as constp,
            tc.tile_pool(name="adj", bufs=6) as adjp,
            tc.tile_pool(name="exg", bufs=4) as exgp,
            tc.tile_pool(name="exb", bufs=2 * EXG + 4) as exbp,
            tc.tile_pool(name="outb", bufs=3) as outbp,
            tc.tile_pool(name="tmp", bufs=3) as tmpp,
            tc.tile_pool(name="ps", bufs=3, space=bass.MemorySpace.PSUM) as psp,
            tc.tile_pool(name="psq", bufs=1, space=bass.MemorySpace.PSUM) as psq,
        ):
            # PE warm-up: the HAM clock gate needs ~3.4us of activity to lift
            # the 1.2GHz cold throttle; burn it on a zero tile while the
            # startup DMAs are in flight.
            # single accumulation group: no inter-MM completion semaphores,
            # so warmup matmuls run back-to-back at the issue rate
            warm_sb = constp.tile([P, P], F16)
            nc.vector.memset(warm_sb[:], 0.0)
            N_WARM = 44
            pwu = psq.tile([P, P], F32, tag="pw")
            for i in range(N_WARM):
                nc.tensor.matmul(pwu[:], warm_sb[:], warm_sb[:],
                                 start=(i == 0), stop=(i == N_WARM - 1))

            # --- startup DMAs in PE-consumption order, kcWh in quarters,
            # so every transfer lands just before its first consumer:
            # [w12, exg0] -> ex0-3, [exg1, adj0] -> ex4-7/spmm0,
            # [kcwQ1..Q4] -> spmm0 k-chunks, [adj1..adj3] -> spmm1-3.
            ex_gs = {}
            eg0 = exgp.tile([P, EXG, KHI_IN, P], F16, tag="exg")
            ex_gs[0] = eg0
            nc.sync.dma_start(eg0[:], exsg[0])
            w12_sb = constp.tile([P, KHI_IN, FB], F16)
            nc.scalar.dma_start(w12_sb[:], w12[:])
            eg1 = exgp.tile([P, EXG, KHI_IN, P], F16, tag="exg")
            ex_gs[1] = eg1
            nc.sync.dma_start(eg1[:], exsg[1])
            adj_sbs = {}
            for t in range(ADJ_PRE):
                ag = adjp.tile([P, KHI_ADJ, P], F16, tag="adj")
                adj_sbs[t] = ag
            nc.scalar.dma_start(adj_sbs[0][:], adjs[0])
            kcw_sb = constp.tile([P, KHI_ADJ, FB], F16)
            QK = KHI_ADJ // 4
            for q in range(4):
                sl = slice(q * QK, (q + 1) * QK)
                ring(q).dma_start(kcw_sb[:, sl, :], kcw[:, sl, :])
            nc.sync.dma_start(adj_sbs[1][:], adjs[1])
            nc.scalar.dma_start(adj_sbs[2][:], adjs[2])
            nc.sync.dma_start(adj_sbs[3][:], adjs[3])

            exb_sbs = {}

            def ex_branch(t):
                """ex_h @ W12 for tile t -> fp16 SBUF tile (via PSUM copy)."""
                eg = ex_gs[t // EXG]
                ps_e = psp.tile([P, FB], F32, tag="pse")
                for kj in range(KHI_IN):
                    nc.tensor.matmul(
                        ps_e[:],
                        eg[:, t % EXG, kj, :],
                        w12_sb[:, kj, :],
                        start=(kj == 0),
                        stop=(kj == KHI_IN - 1),
                    )
                # copy on the vector engine: the scalar engine's DMA issues
                # must never delay freeing the ex-branch PSUM buffer
                exb = exbp.tile([P, FB], F16)
                nc.vector.tensor_copy(exb[:], ps_e[:])
                exb_sbs[t] = exb

            def elementwise(t, ps_s, exb_ap, o_sb, n):
                # elu(prod) = max(prod, exp(min(prod,0)) - 1)
                prod = tmpp.tile([P, n], F32, tag=f"prod{n}")
                nc.vector.tensor_tensor(prod[:], ps_s[:], exb_ap, OP.mult)
                nmin = tmpp.tile([P, n], F32, tag=f"nmin{n}")
                nc.vector.tensor_scalar(nmin[:], prod[:], 0.0, None, OP.min)
                expv = tmpp.tile([P, n], F32, tag=f"expv{n}")
                nc.scalar.activation(expv[:], nmin[:], AF.Exp)
                nc.vector.scalar_tensor_tensor(
                    o_sb, expv[:], -1.0, prod[:], OP.add, OP.max)

            # ex branches of the first two groups run while kcWh is in flight
            for t in range(2 * EXG):
                ex_branch(t)

            # ---- main loop over row-tiles ----
            for t in range(n_tiles):
                if t >= 2 * EXG:
                    ex_branch(t)

                a_sb = adj_sbs.pop(t)
                exb = exb_sbs.pop(t)
                if t < n_tiles - 1:
                    ps_s = psp.tile([P, FB], F32, tag="pss")  # spmm branch
                    for ki in range(KHI_ADJ):
                        nc.tensor.matmul(
                            ps_s[:],
                            a_sb[:, ki, :],
                            kcw_sb[:, ki, :],
                            start=(ki == 0),
                            stop=(ki == KHI_ADJ - 1),
                        )
                    o_sb = outbp.tile([P, FB], F16)
                    elementwise(t, ps_s, exb[:], o_sb[:], FB)
                    # lookahead DMA issues BEFORE the store: their buffers
                    # are released tiles ago (pool depth > lookahead), so
                    # they issue immediately instead of queuing behind the
                    # store's wait on the elementwise chain
                    ta = t + ADJ_PRE
                    if ta < n_tiles:
                        ag = adjp.tile([P, KHI_ADJ, P], F16, tag="adj")
                        ring(ta).dma_start(ag[:], adjs[ta])
                        adj_sbs[ta] = ag
                    if t % EXG == 1:
                        g = t // EXG + 2
                        if g < (n_tiles + EXG - 1) // EXG:
                            eg = exgp.tile([P, EXG, KHI_IN, P], F16)
                            ring(g).dma_start(eg[:], exsg[g])
                            ex_gs[g] = eg
                    ring(t).dma_start(outp[t], o_sb[:])
                else:
                    # last tile: split N in half so the elementwise chain and
                    # store of half 0 overlap the spmm matmuls of half 1
                    HF = FB // 2
                    for h in range(2):
                        sl = slice(h * HF, (h + 1) * HF)
                        ps_h = psp.tile([P, HF], F32, tag="pss")
                        for ki in range(KHI_ADJ):
                            nc.tensor.matmul(
                                ps_h[:],
                                a_sb[:, ki, :],
                                kcw_sb[:, ki, sl],
                                start=(ki == 0),
                                stop=(ki == KHI_ADJ - 1),
                            )
                        o_sb = outbp.tile([P, HF], F16, tag="olast")
                        elementwise(t, ps_h, exb[:, sl], o_sb[:], HF)
                        ring(h).dma_start(outp[t, :, sl], o_sb[:])

    nc.compile()
    return nc


def prep_inputs(exercise_h, kc_h, adj_exercise_kc, W1, W2,
                n_tiles: int = T):
    """Host-side shard + layout prep. Returns in_maps (one dict per core)."""
    ex = np.asarray(exercise_h, dtype=np.float32)
    kc = np.asarray(kc_h, dtype=np.float32)
    adj = np.asarray(adj_exercise_kc, dtype=np.float32)
    w1 = np.asarray(W1, dtype=np.float32)
    w2 = np.asarray(W2, dtype=np.float32)

    # batch-independent weight folding (exact fp32, then one fp16 rounding)
    kcwh = (kc @ w1).astype(np.float16)                    # [2048, 512]
    w12 = (w1 @ w2).astype(np.float16)                     # [512, 512]

    e_pad = N_CORES * n_tiles * P
    n_rows = min(N_EX, e_pad)

    adj_p = np.zeros((e_pad, N_KC), np.float16)
    adj_p[:n_rows] = adj[:n_rows]
    n_exg = (n_tiles + EXG - 1) // EXG
    ex_p = np.zeros((N_CORES * n_exg * EXG * P, IN_F), np.float16)
    # per-core row blocks padded to the group grid
    exf = np.asarray(ex[:n_rows], np.float16)
    for c in range(N_CORES):
        lo, hi = c * E_PER_CORE, min((c + 1) * E_PER_CORE, n_rows)
        ex_p[c * n_exg * EXG * P: c * n_exg * EXG * P + (hi - lo)] = \
            exf[lo:hi]
    # [core, t, k_lo, k_hi, m] (k-major chunks, stationary-operand layout)
    adjs = np.ascontiguousarray(
        adj_p.reshape(N_CORES, n_tiles, P, KHI_ADJ, P).transpose(0, 1, 4, 3, 2))
    # [core, group, k_lo, tile-in-group, kj, m]
    exsg = np.ascontiguousarray(
        ex_p.reshape(N_CORES, n_exg, EXG, P, KHI_IN, P)
        .transpose(0, 1, 5, 2, 4, 3))

    kcw = np.ascontiguousarray(
        kcwh.reshape(KHI_ADJ, P, FB).transpose(1, 0, 2))
    w12r = np.ascontiguousarray(w12.reshape(KHI_IN, P, FB).transpose(1, 0, 2))

    return [
        {"adjs": adjs[c], "exsg": exsg[c], "kcw": kcw, "w12": w12r}
        for c in range(N_CORES)
    ]


def unpack_output(results, n_tiles: int = T) -> np.ndarray:
    """results: list per core of {"outp": [n_tiles, P, FB]} -> [N_EX, FB]."""
    per_core = [
        np.asarray(r["outp"]).reshape(n_tiles * P, FB)
        for r in results
    ]
    return np.concatenate(per_core, axis=0)[:N_EX].astype(np.float32)


_NC_CACHE: dict = {}


def _get_nc():
    if T not in _NC_CACHE:
        _NC_CACHE[T] = build_nc()
    return _NC_CACHE[T]


def kernel(exercise_h, kc_h, adj_exercise_kc, W1, W2):
    nc = _get_nc()
    in_maps = prep_inputs(exercise_h, kc_h, adj_exercise_kc, W1, W2)
    res = run_bass_kernel_spmd(nc, in_maps, core_ids=list(range(N_CORES)))
    return np.ascontiguousarray(unpack_output(res.results))
